# revision 1
# baseline (speedup 1.0000x reference)
"""Bass/Tile TRN2 kernel for nn_BertSelfAttention2 (B=2, S=2048, D=1024, H=16).

Sharding: 8 cores = 2 (batch) x 4 (head groups of 4 heads). Each core
computes Q/K projections for its 4 heads (as 2 packed pairs), the modified
attention (kt = softplus(k), v = q + k, mask on the query axis), and writes
its [S, 256] slice of the output.

Layout trick: everything is computed in "T" orientation (scoresT[k, q]) so
no large on-device transposes are needed. The query-axis mask is applied by
zeroing masked query columns of Q; softmax of an all-zero score column then
reproduces the reference's uniform-probability behaviour for masked queries
exactly. The softmax denominator comes from a ones-column appended to V.
"""
import sys

if "/opt/trn_rl_repo" not in sys.path:
    sys.path.insert(0, "/opt/trn_rl_repo")

import numpy as np

B, S, D = 2, 2048, 1024
H = 16
HD = 64
NCORES = 8
HPC = H // (NCORES // B)     # heads per core = 4
NG = HPC // 2                # head-pair groups per core = 2
SC = 4                       # 512-wide query chunks
KC = S // 128                # 16 key chunks
SUPER = 2                    # key chunks per exp supertile

_CACHE = {}


def _build():
    import concourse.tile as tile
    from concourse import bacc, mybir
    from concourse.masks import make_identity

    F32 = mybir.dt.float32
    F32R = mybir.dt.float32r
    AF = mybir.ActivationFunctionType

    nc = bacc.Bacc(None, target_bir_lowering=False, debug=False)

    # all tiled operands are shipped pre-tiled so every SBUF tile load is
    # one contiguous DRAM read (strided row reads leave the DMA engines
    # descriptor-bound at ~1/3 utilization)
    xt = nc.declare_dram_parameter("xt", [SC * 8 * 128, 512], F32R, isOutput=False)
    wq = nc.declare_dram_parameter("wq", [NG * 8 * 128, 128], F32R, isOutput=False)
    wk = nc.declare_dram_parameter("wk", [NG * 8 * 128, 128], F32R, isOutput=False)
    bq = nc.declare_dram_parameter("bq", [2 * 128], F32, isOutput=False)
    bk = nc.declare_dram_parameter("bk", [2 * 128], F32, isOutput=False)
    maskb = nc.declare_dram_parameter("maskb", [1, S], F32, isOutput=False)
    out = nc.declare_dram_parameter("out", [NG * S, 128], F32, isOutput=True)

    with tile.TileContext(nc) as tc:
        with tc.tile_pool(name="consts", bufs=1) as consts, \
             tc.tile_pool(name="big", bufs=1) as big, \
             tc.tile_pool(name="tmp", bufs=2) as tmp, \
             tc.tile_pool(name="expp", bufs=2) as expp, \
             tc.tile_pool(name="ep", bufs=2) as ep, \
             tc.tile_pool(name="ps_s", bufs=1, space="PSUM") as ps_s, \
             tc.tile_pool(name="ps_c", bufs=1, space="PSUM") as ps_c, \
             tc.tile_pool(name="ps_m", bufs=2, space="PSUM") as ps_m:

            ident = consts.tile([128, 128], F32)
            make_identity(nc, ident)

            # input loads round-robin over the three DMA-capable sequencers;
            # a single sequencer takes ~565ns per dma_start issue and becomes
            # the startup critical path
            _dmaeng = [nc.sync]
            _dmactr = [0]

            def ldma(out, in_):
                e = _dmaeng[0]
                _dmactr[0] += 1
                e.dma_start(out=out, in_=in_)

            # group-0 weights + small consts first, then X^T column-major so
            # the first projection chunk is ready ASAP.
            wq_t = [[consts.tile([128, 128], F32R, tag=f"wq{g}_{dchunk}",
                                 name=f"wq{g}_{dchunk}")
                     for dchunk in range(8)] for g in range(NG)]
            wk_t = [[consts.tile([128, 128], F32R, tag=f"wk{g}_{dchunk}",
                                 name=f"wk{g}_{dchunk}")
                     for dchunk in range(8)] for g in range(NG)]
            for dc in range(8):
                nc.scalar.dma_start(out=wq_t[0][dc],
                                    in_=wq[dc * 128:(dc + 1) * 128, :])
                nc.scalar.dma_start(out=wk_t[0][dc],
                                    in_=wk[dc * 128:(dc + 1) * 128, :])

            bq_t, bk_t = [], []
            for g in range(NG):
                bqt = consts.tile([128, 1], F32, tag=f"bq{g}", name=f"bq{g}")
                nc.scalar.dma_start(
                    out=bqt,
                    in_=bq[g * 128:(g + 1) * 128].rearrange("(p o) -> p o", o=1))
                bq_t.append(bqt)
                bkt = consts.tile([128, 1], F32, tag=f"bk{g}", name=f"bk{g}")
                nc.scalar.dma_start(
                    out=bkt,
                    in_=bk[g * 128:(g + 1) * 128].rearrange("(p o) -> p o", o=1))
                bk_t.append(bkt)

            # X^T as 32 [128, 512] tiles, loaded s-chunk-major; sc=0 first
            # (with the mask chunk it needs) so the first projection can
            # start as early as possible.
            xt_t = [[big.tile([128, 512], F32R, tag=f"xt{dchunk}_{scc}",
                              name=f"xt{dchunk}_{scc}")
                     for scc in range(SC)] for dchunk in range(8)]
            mask_t = [consts.tile([128, 512], F32, tag=f"mask{scc}",
                                  name=f"mask{scc}") for scc in range(SC)]
            mask_row = consts.tile([1, S], F32)
            nc.scalar.dma_start(out=mask_row, in_=maskb[:, :])
            for scc in range(SC):
                nc.gpsimd.partition_broadcast(
                    mask_t[scc], mask_row[0:1, scc * 512:(scc + 1) * 512])
                for dc in range(8):
                    base = (scc * 8 + dc) * 128
                    nc.sync.dma_start(out=xt_t[dc][scc],
                                      in_=xt[base:base + 128, :])

            for dc in range(8):
                base = (8 + dc) * 128
                nc.gpsimd.dma_start(out=wq_t[1][dc], in_=wq[base:base + 128, :])
                nc.gpsimd.dma_start(out=wk_t[1][dc], in_=wk[base:base + 128, :])

            # persistent activations, split into per-chunk tiles so the
            # scheduler sees fine-grained dependencies.
            # qt is stored twice with the other head's rows zeroed so the
            # scores matmuls run with a full K=128 contraction (reduced-K
            # fp32r matmuls do not register as PE activity for HAM and the
            # clock throttles to 1.2GHz). kt is the shared stationary.
            qtp = [[[big.tile([128, 512], F32R, tag=f"qtp{g}_{hh}_{scc}",
                              name=f"qtp{g}_{hh}_{scc}") for scc in range(SC)]
                    for hh in range(2)] for g in range(NG)]
            kt = [[big.tile([128, 512], F32R, tag=f"kt{g}_{scc}",
                            name=f"kt{g}_{scc}") for scc in range(SC)]
                  for g in range(NG)]
            vp = [[big.tile([128, 65], F32R, tag=f"vp{h}_{kc}",
                            name=f"vp{h}_{kc}") for kc in range(KC)]
                  for h in range(HPC)]

            def emit_vtrans(g, sc, vts):
                for hh in range(2):
                    h = g * 2 + hh
                    hsl = slice(hh * 64, (hh + 1) * 64)
                    for jj in range(4):
                        j = sc * 4 + jj
                        # g=0: ctx banks are idle during proj(0); keep the
                        # two ep slots free for the Q/K psum accumulators
                        pv = (ps_c.tile([128, 65], F32, tag="cA" if hh == 0
                                        else "cB", name=f"pv{g}_{hh}_{j}")
                              if g == 0 else
                              ps_m.tile([128, 65], F32, tag="ep",
                                        name=f"pv{g}_{hh}_{j}"))
                        nc.tensor.transpose(pv[:, 0:64],
                                            vts[hsl, jj * 128:(jj + 1) * 128],
                                            ident[hsl, hsl])
                        nc.vector.memset(pv[:, 64:65], 1.0)
                        nc.vector.tensor_copy(vp[h][j], pv)

            def proj_group(g, scs=None, vts_hist=None):
                if vts_hist is None:
                    vts_hist = []
                for sc in (range(SC) if scs is None else scs):
                    ssl = slice(sc * 512, (sc + 1) * 512)
                    pq = ps_m.tile([128, 512], F32, tag="ep", name=f"pq{g}_{sc}")
                    for dc in range(8):
                        nc.tensor.matmul(pq[:, 0:512], wq_t[g][dc],
                                         xt_t[dc][sc],
                                         start=(dc == 0), stop=(dc == 7))
                    pk = ps_m.tile([128, 512], F32, tag="ep", name=f"pk{g}_{sc}")
                    for dc in range(8):
                        nc.tensor.matmul(pk[:, 0:512], wk_t[g][dc],
                                         xt_t[dc][sc],
                                         start=(dc == 0), stop=(dc == 7))
                    tq = tmp.tile([128, 512], F32, tag="tq", name=f"tq{g}_{sc}")
                    nc.vector.tensor_scalar_add(tq, pq[:, 0:512], bq_t[g])
                    tk = tmp.tile([128, 512], F32, tag="tk", name=f"tk{g}_{sc}")
                    nc.vector.tensor_scalar_add(tk, pk[:, 0:512], bk_t[g])
                    # v = q + k (raw)
                    vts = tmp.tile([128, 512], F32, tag="vts", name=f"vts{g}_{sc}")
                    nc.vector.tensor_add(vts, tq, tk)
                    # masked q for scores, split per head into the
                    # zero-padded stores
                    nc.vector.tensor_mul(qtp[g][0][sc][0:64, :], tq[0:64, :],
                                         mask_t[sc][0:64, :])
                    nc.vector.tensor_scalar_mul(qtp[g][0][sc][64:128, :],
                                                tq[64:128, :], 0.0)
                    nc.vector.tensor_mul(qtp[g][1][sc][64:128, :], tq[64:128, :],
                                         mask_t[sc][64:128, :])
                    nc.vector.tensor_scalar_mul(qtp[g][1][sc][0:64, :],
                                                tq[0:64, :], 0.0)
                    # kt = softplus(k) = ln(exp(k) + 1)
                    te = tmp.tile([128, 512], F32, tag="tq", name=f"te{g}_{sc}")
                    nc.scalar.activation(out=te, in_=tk, func=AF.Exp)
                    nc.scalar.activation(out=kt[g][sc], in_=te,
                                         func=AF.Ln, bias=1.0)
                    vts_hist.append(vts)
                    # V' transposes run one s-chunk behind so the PE is never
                    # gated on this chunk's DVE chain
                    if sc > 0:
                        emit_vtrans(g, sc - 1, vts_hist[sc - 1])
                    if sc == SC - 1:
                        emit_vtrans(g, sc, vts_hist[sc])
                return vts_hist

            def attn_group(g, qcs=None):
                vpA = vp[g * 2]
                vpB = vp[g * 2 + 1]
                ktg = kt[g]
                for qc in (range(SC) if qcs is None else qcs):
                    qsl = slice(qc * 512, (qc + 1) * 512)
                    cA = ps_c.tile([65, 512], F32, tag="cA", name=f"cA{g}_{qc}")
                    cB = ps_c.tile([65, 512], F32, tag="cB", name=f"cB{g}_{qc}")
                    for st in range(KC // SUPER):
                        sA = ps_s.tile([128, SUPER * 512], F32, tag="sA",
                                       name=f"sA{g}_{qc}_{st}")
                        sB = ps_s.tile([128, SUPER * 512], F32, tag="sB",
                                       name=f"sB{g}_{qc}_{st}")
                        for kk in range(SUPER):
                            kc = st * SUPER + kk
                            osl = slice(kk * 512, (kk + 1) * 512)
                            lhs = ktg[kc // 4][:, (kc % 4) * 128:
                                                  (kc % 4 + 1) * 128]
                            nc.tensor.matmul(sA[:, osl], lhs,
                                             qtp[g][0][qc],
                                             start=True, stop=True)
                            nc.tensor.matmul(sB[:, osl], lhs,
                                             qtp[g][1][qc],
                                             start=True, stop=True)
                        eA = expp.tile([128, SUPER * 512], F32R, tag="eA",
                                       name=f"eA{g}_{qc}_{st}")
                        nc.scalar.activation(out=eA, in_=sA, func=AF.Exp,
                                             scale=0.125)
                        eB = expp.tile([128, SUPER * 512], F32R, tag="eB",
                                       name=f"eB{g}_{qc}_{st}")
                        nc.scalar.activation(out=eB, in_=sB, func=AF.Exp,
                                             scale=0.125)
                        for kk in range(SUPER):
                            kc = st * SUPER + kk
                            osl = slice(kk * 512, (kk + 1) * 512)
                            nc.tensor.matmul(cA, vpA[kc], eA[:, osl],
                                             start=(kc == 0), stop=(kc == KC - 1))
                            nc.tensor.matmul(cB, vpB[kc], eB[:, osl],
                                             start=(kc == 0), stop=(kc == KC - 1))
                    # epilogue: transpose ctxT back, normalize, store
                    csA = ep.tile([65, 512], F32, tag="csA", name=f"csA{g}_{qc}")
                    nc.vector.tensor_copy(csA, cA)
                    csB = ep.tile([65, 512], F32, tag="csB", name=f"csB{g}_{qc}")
                    nc.vector.tensor_copy(csB, cB)
                    for j in range(4):
                        jsl = slice(j * 128, (j + 1) * 128)
                        ptA = (ps_c.tile([128, 65], F32, tag="cA",
                                         name=f"ptA{g}_{qc}_{j}") if g == 0 else
                               ps_m.tile([128, 65], F32, tag="ep",
                                         name=f"ptA{g}_{qc}_{j}"))
                        nc.tensor.transpose(ptA[:, :], csA[:, jsl],
                                            ident[0:65, 0:65])
                        ptB = (ps_c.tile([128, 65], F32, tag="cB",
                                         name=f"ptB{g}_{qc}_{j}") if g == 0 else
                               ps_m.tile([128, 65], F32, tag="ep",
                                         name=f"ptB{g}_{qc}_{j}"))
                        nc.tensor.transpose(ptB[:, :], csB[:, jsl],
                                            ident[0:65, 0:65])
                        rA = ep.tile([128, 1], F32, tag="rA", name=f"rA{g}_{qc}_{j}")
                        nc.vector.reciprocal(rA, ptA[:, 64:65])
                        rB = ep.tile([128, 1], F32, tag="rB", name=f"rB{g}_{qc}_{j}")
                        nc.vector.reciprocal(rB, ptB[:, 64:65])
                        cf = ep.tile([128, 128], F32, tag="cf", name=f"cf{g}_{qc}_{j}")
                        nc.vector.tensor_scalar_mul(cf[:, 0:64], ptA[:, 0:64], rA)
                        nc.vector.tensor_scalar_mul(cf[:, 64:128], ptB[:, 0:64], rB)
                        row = g * S + qc * 512 + j * 128
                        nc.sync.dma_start(out=out[row:row + 128, :], in_=cf)

            for g in range(NG):
                proj_group(g)
                attn_group(g)

    nc.finalize()
    return nc


def _get_nc():
    if "nc" not in _CACHE:
        _CACHE["nc"] = _build()
    return _CACHE["nc"]


def _shard_inputs(hidden_states, attention_mask, Wq, bq, Wk, bk):
    hs = np.asarray(hidden_states, dtype=np.float32)
    am = np.asarray(attention_mask)
    Wq = np.asarray(Wq, dtype=np.float32)
    Wk = np.asarray(Wk, dtype=np.float32)
    bq = np.asarray(bq, dtype=np.float32)
    bk = np.asarray(bk, dtype=np.float32)

    xts = [np.ascontiguousarray(
        hs[b].T.reshape(8, 128, SC, 512).transpose(2, 0, 1, 3)
        .reshape(SC * 8 * 128, 512)) for b in range(B)]
    maskbs = [np.ascontiguousarray(am[b].astype(np.float32)[None, :])
              for b in range(B)]

    in_maps = []
    for c in range(NCORES):
        b = c // (NCORES // B)
        hg = c % (NCORES // B)
        cols = slice(hg * 2 * 128, (hg + 1) * 2 * 128)
        def _tile_w(W):
            return np.ascontiguousarray(
                W[:, cols].reshape(8, 128, NG, 128).transpose(2, 0, 1, 3)
                .reshape(NG * 8 * 128, 128))
        in_maps.append({
            "xt": xts[b],
            "wq": _tile_w(Wq),
            "wk": _tile_w(Wk),
            "bq": np.ascontiguousarray(bq[cols]),
            "bk": np.ascontiguousarray(bk[cols]),
            "maskb": maskbs[b],
        })
    return in_maps


def _gather(results):
    full = np.empty((B, S, D), dtype=np.float32)
    for c in range(NCORES):
        b = c // (NCORES // B)
        hg = c % (NCORES // B)
        cols = slice(hg * 2 * 128, (hg + 1) * 2 * 128)
        r = results[c]["out"].reshape(NG, S, 128)
        full[b, :, cols] = np.concatenate([r[0], r[1]], axis=1)
    return full


def run_sharded(in_maps, **kw):
    from concourse.bass_utils import run_bass_kernel_spmd
    nc = _get_nc()
    return run_bass_kernel_spmd(nc, in_maps, list(range(NCORES)), **kw)


def kernel(hidden_states, attention_mask, Wq, bq, Wk, bk):
    in_maps = _shard_inputs(hidden_states, attention_mask, Wq, bq, Wk, bk)
    res = run_sharded(in_maps)
    return _gather(res.results)



# revision 14
# speedup vs baseline: 1.0184x; 1.0184x over previous
"""Bass/Tile TRN2 kernel for nn_BertSelfAttention2 (B=2, S=2048, D=1024, H=16).

Sharding: 8 cores = 2 (batch) x 4 (head groups of 4 heads). Each core
computes Q/K projections for its 4 heads (as 2 packed pairs), the modified
attention (kt = softplus(k), v = q + k, mask on the query axis), and writes
its slice of the output.

Everything is computed in "T" orientation (scoresT[k, q]) so no large
on-device transposes are needed.

The query-axis mask is exploited rather than applied: every masked query
produces the SAME output (uniform softmax over an all-zero score column =
mean(v)/1), so only the unmasked query columns are gathered (gpsimd
ap_gather) into a packed set, plus one representative zeroed column whose
result the host broadcasts to all masked positions.  With ~50% masked
queries this cuts scores/exp/ctx work by 25% (4x512 -> 4x384 packed cols).

All matmul operands are bf16 (fp32r streams ~2.3 cycles/col on TRN2's PE,
bf16 streams 1); accumulation stays fp32 in PSUM.  kt = softplus(k) is a
single activation (avoids Exp<->Ln act-table thrash).  The softmax
normalization happens in the T layout: reciprocal of the denominator row,
broadcast down 64 partitions via a K=1 PE outer product, one DVE multiply,
then DMA out; the final [head*64, S] -> [S, 256] transpose is host-side.
"""
import sys

if "/opt/trn_rl_repo" not in sys.path:
    sys.path.insert(0, "/opt/trn_rl_repo")

import numpy as np
import ml_dtypes

B, S, D = 2, 2048, 1024
H = 16
HD = 64
NCORES = 8
HPC = H // (NCORES // B)     # heads per core = 4
NG = HPC // 2                # head-pair groups per core = 2
SC = 4                       # 512-wide seq chunks
KC = S // 128                # 16 key chunks

_CACHE = {}
_META = {}


def _build(cap, zero_col0):
    """cap: packed query columns kept per 512-seq chunk (must be %4==0 and
    %16==0 for the wrapped idx layout; 384 normally, 512 fallback).
    zero_col0: zero packed column 0 (the masked-query representative)."""
    import concourse.tile as tile
    from concourse import bacc, mybir

    F32 = mybir.dt.float32
    F32R = mybir.dt.float32r
    BF16 = mybir.dt.bfloat16
    I16 = mybir.dt.int16
    AF = mybir.ActivationFunctionType

    NQ = cap * SC                # packed query columns (1536 normally)
    SQ = NQ // 512               # packed 512-col chunks (3 normally)
    assert NQ % 512 == 0 and cap % 16 == 0

    nc = bacc.Bacc(None, target_bir_lowering=False, debug=False)

    # all tiled operands are shipped pre-tiled so every SBUF tile load is
    # one contiguous DRAM read
    xt = nc.declare_dram_parameter("xt", [SC * 8 * 128, 512], BF16, isOutput=False)
    wq = nc.declare_dram_parameter("wq", [NG * 8 * 128, 128], BF16, isOutput=False)
    wk = nc.declare_dram_parameter("wk", [NG * 8 * 128, 128], BF16, isOutput=False)
    bq = nc.declare_dram_parameter("bq", [2 * 128], F32, isOutput=False)
    bk = nc.declare_dram_parameter("bk", [2 * 128], F32, isOutput=False)
    qidx = nc.declare_dram_parameter("qidx", [SC * 128, cap // 16], I16,
                                     isOutput=False)
    out = nc.declare_dram_parameter("out", [HPC * HD, NQ], F32, isOutput=True)

    with tile.TileContext(nc) as tc:
        with tc.tile_pool(name="consts", bufs=1) as consts, \
             tc.tile_pool(name="big", bufs=1) as big, \
             tc.tile_pool(name="tmp", bufs=2) as tmp, \
             tc.tile_pool(name="expp", bufs=3) as expp, \
             tc.tile_pool(name="ep", bufs=2) as ep, \
             tc.tile_pool(name="ps_s", bufs=2, space="PSUM") as ps_s, \
             tc.tile_pool(name="ps_c", bufs=1, space="PSUM") as ps_c, \
             tc.tile_pool(name="ps_m", bufs=2, space="PSUM") as ps_m:

            # ---- constant / input loads, spread across the DMA-capable
            # sequencers so no single issue queue gates startup ----
            wq_t = [[consts.tile([128, 128], BF16, tag=f"wq{g}_{dc}",
                                 name=f"wq{g}_{dc}") for dc in range(8)]
                    for g in range(NG)]
            wk_t = [[consts.tile([128, 128], BF16, tag=f"wk{g}_{dc}",
                                 name=f"wk{g}_{dc}") for dc in range(8)]
                    for g in range(NG)]
            xt_t = [[big.tile([128, 512], BF16, tag=f"xt{dc}_{scc}",
                              name=f"xt{dc}_{scc}") for scc in range(SC)]
                    for dc in range(8)]
            qidx_t = [consts.tile([128, cap // 16], I16, tag=f"qi{scc}",
                                  name=f"qi{scc}") for scc in range(SC)]

            # first projection chunk needs wq g0 + xt sc0 + bq
            for dc in range(8):
                nc.sync.dma_start(out=wq_t[0][dc],
                                  in_=wq[dc * 128:(dc + 1) * 128, :])
                nc.gpsimd.dma_start(out=xt_t[dc][0],
                                    in_=xt[dc * 128:(dc + 1) * 128, :])
                nc.scalar.dma_start(out=wk_t[0][dc],
                                    in_=wk[dc * 128:(dc + 1) * 128, :])
            bq_t, bk_t = [], []
            for g in range(NG):
                bqt = consts.tile([128, 1], F32, tag=f"bq{g}", name=f"bq{g}")
                nc.scalar.dma_start(
                    out=bqt,
                    in_=bq[g * 128:(g + 1) * 128].rearrange("(p o) -> p o", o=1))
                bq_t.append(bqt)
                bkt = consts.tile([128, 1], F32, tag=f"bk{g}", name=f"bk{g}")
                nc.scalar.dma_start(
                    out=bkt,
                    in_=bk[g * 128:(g + 1) * 128].rearrange("(p o) -> p o", o=1))
                bk_t.append(bkt)
            for scc in range(SC):
                nc.gpsimd.dma_start(out=qidx_t[scc],
                                    in_=qidx[scc * 128:(scc + 1) * 128, :])

            # remaining xt chunks + group-1 weights
            rr = [nc.sync, nc.gpsimd, nc.scalar]
            ri = 0
            for scc in range(1, SC):
                for dc in range(8):
                    base = (scc * 8 + dc) * 128
                    rr[ri % 3].dma_start(out=xt_t[dc][scc],
                                         in_=xt[base:base + 128, :])
                    ri += 1
            for dc in range(8):
                base = (8 + dc) * 128
                nc.gpsimd.dma_start(out=wq_t[1][dc], in_=wq[base:base + 128, :])
                nc.sync.dma_start(out=wk_t[1][dc], in_=wk[base:base + 128, :])

            # identity for the small V transposes (f32r: 1.5 cyc/row)
            from concourse.masks import make_identity
            ident = consts.tile([128, 128], F32)
            make_identity(nc, ident)

            # ---- persistent activations ----
            # tqf: q + bias, full seq, fp32 (gather source)
            tqf = [big.tile([128, S], F32, tag=f"tqf{g}", name=f"tqf{g}")
                   for g in range(NG)]
            # qg: gathered packed q, fp32
            qg = [big.tile([128, NQ], F32, tag=f"qg{g}", name=f"qg{g}")
                  for g in range(NG)]
            # qtp: packed q per head, bf16, other head's rows zero so the
            # scores matmuls run with a full K=128 contraction
            qtp = [[big.tile([128, NQ], BF16, tag=f"qtp{g}_{hh}",
                             name=f"qtp{g}_{hh}") for hh in range(2)]
                   for g in range(NG)]
            for g in range(NG):
                nc.vector.memset(qtp[g][0][64:128, :], 0.0)
                nc.vector.memset(qtp[g][1][0:64, :], 0.0)
            # kt = softplus(k), both heads packed on partitions
            kt = [[big.tile([128, 512], BF16, tag=f"kt{g}_{sc}",
                            name=f"kt{g}_{sc}") for sc in range(SC)]
                  for g in range(NG)]
            # vp: v^T per head per key chunk, with a ones column for the
            # softmax denominator
            vp = [[big.tile([128, 65], BF16, tag=f"vp{h}_{kc}",
                            name=f"vp{h}_{kc}") for kc in range(KC)]
                  for h in range(HPC)]

            def emit_vtrans(g, sc, vts):
                # one [128,128] PE transpose yields both heads' v^T slices
                for jj in range(4):
                    j = sc * 4 + jj
                    pv = ps_m.tile([128, 128], F32, tag="ep",
                                   name=f"pv{g}_{j}")
                    nc.tensor.transpose(pv, vts[:, jj * 128:(jj + 1) * 128],
                                        ident)
                    for hh in range(2):
                        h = g * 2 + hh
                        nc.vector.tensor_copy(vp[h][j][:, 0:64],
                                              pv[:, hh * 64:(hh + 1) * 64])
                        nc.vector.memset(vp[h][j][:, 64:65], 1.0)

            def proj_group(g, pending):
                # fire the previous attn group's last epilogue first (it is
                # DVE/gpsimd/DMA-only) so its reads of cA/cB precede the next
                # attn group's reuse of those banks
                if pending[0] is not None:
                    pending[0]()
                    pending[0] = None
                vts_hist = []
                te_hist = []
                for sc in range(SC):
                    ssl = slice(sc * 512, (sc + 1) * 512)
                    csl = slice(sc * cap, (sc + 1) * cap)
                    pq = ps_m.tile([128, 512], F32, tag="ep", name=f"pq{g}_{sc}")
                    for dc in range(8):
                        nc.tensor.matmul(pq[:, 0:512], wq_t[g][dc],
                                         xt_t[dc][sc],
                                         start=(dc == 0), stop=(dc == 7))
                    pk = ps_m.tile([128, 512], F32, tag="ep", name=f"pk{g}_{sc}")
                    for dc in range(8):
                        nc.tensor.matmul(pk[:, 0:512], wk_t[g][dc],
                                         xt_t[dc][sc],
                                         start=(dc == 0), stop=(dc == 7))
                    nc.vector.tensor_scalar_add(tqf[g][:, ssl], pq[:, 0:512],
                                                bq_t[g])
                    tk = tmp.tile([128, 512], F32, tag="tk", name=f"tk{g}_{sc}")
                    nc.vector.tensor_scalar_add(tk, pk[:, 0:512], bk_t[g])
                    # kt = softplus(k) = ln(exp(k) + 1).  The Exps run per-sc
                    # but the Lns are batched after the loop so the scalar
                    # engine switches act tables twice per group, not per sc.
                    te = tmp.tile([128, 512], F32, tag=f"te{sc}",
                                  name=f"te{g}_{sc}")
                    nc.scalar.activation(out=te, in_=tk, func=AF.Exp)
                    te_hist.append(te)
                    # v = q + k (raw, f32r so the PE transpose is cheap)
                    vts = tmp.tile([128, 512], F32, tag="vts",
                                   name=f"vts{g}_{sc}")
                    nc.vector.tensor_add(vts, tqf[g][:, ssl], tk)
                    vts_hist.append(vts)
                    # packed-q gather for this seq chunk (gpsimd, idle engine)
                    nc.gpsimd.ap_gather(
                        qg[g][:, csl], tqf[g][:, ssl], qidx_t[sc],
                        channels=128, num_elems=512, d=1, num_idxs=cap)
                    nc.vector.tensor_copy(qtp[g][0][0:64, csl],
                                          qg[g][0:64, csl])
                    nc.vector.tensor_copy(qtp[g][1][64:128, csl],
                                          qg[g][64:128, csl])
                    # V' transposes run one s-chunk behind so the PE is never
                    # gated on this chunk's DVE chain
                    if sc > 0:
                        emit_vtrans(g, sc - 1, vts_hist[sc - 1])
                    if sc == SC - 1:
                        emit_vtrans(g, sc, vts_hist[sc])
                for sc in range(SC):
                    nc.scalar.activation(out=kt[g][sc], in_=te_hist[sc],
                                         func=AF.Ln, bias=1.0)
                if zero_col0:
                    nc.vector.memset(qtp[g][0][0:64, 0:1], 0.0)
                    nc.vector.memset(qtp[g][1][64:128, 0:1], 0.0)

            SQC = NQ // 512      # packed 512-col matmul chunks

            def attn_group(g, pending_tail):
                vpA = vp[g * 2]
                vpB = vp[g * 2 + 1]
                ktg = kt[g]
                pending = [pending_tail]

                def epilogue(qc, cA, cB):
                    # no PE ops: reciprocal of the denominator row (DVE),
                    # broadcast down 64 partitions (gpsimd), multiply, DMA.
                    # Deferred to the next qc's kc==0 (before its first ctx
                    # matmul reuses the single-buffered cA/cB banks).
                    def emit():
                        for hh, c in ((0, cA), (1, cB)):
                            rcp = ep.tile([1, 512], F32, tag=f"r{hh}",
                                          name=f"r{g}_{qc}_{hh}")
                            nc.vector.reciprocal(rcp, c[64:65, :])
                            bc = ep.tile([64, 512], F32, tag=f"b{hh}",
                                         name=f"b{g}_{qc}_{hh}")
                            nc.gpsimd.partition_broadcast(bc, rcp[0:1, :])
                            cf = ep.tile([64, 512], F32, tag=f"cf{hh}",
                                         name=f"cf{g}_{qc}_{hh}")
                            nc.vector.tensor_mul(cf, c[0:64, :], bc)
                            row = (g * 2 + hh) * 64
                            nc.sync.dma_start(
                                out=out[row:row + 64, qc * 512:(qc + 1) * 512],
                                in_=cf)
                    return emit

                for qc in range(SQC):
                    qsl = slice(qc * 512, (qc + 1) * 512)
                    cA = ps_c.tile([65, 512], F32, tag="cA", name=f"cA{g}_{qc}")
                    cB = ps_c.tile([65, 512], F32, tag="cB", name=f"cB{g}_{qc}")
                    prev = None
                    for kc in range(KC):
                        sAB = ps_s.tile([128, 1024], F32, tag="sAB",
                                        name=f"s{g}_{qc}_{kc}")
                        lhs = ktg[kc // 4][:, (kc % 4) * 128:(kc % 4 + 1) * 128]
                        nc.tensor.matmul(sAB[:, 0:512], lhs,
                                         qtp[g][0][:, qsl],
                                         start=True, stop=True)
                        nc.tensor.matmul(sAB[:, 512:1024], lhs,
                                         qtp[g][1][:, qsl],
                                         start=True, stop=True)
                        eAB = expp.tile([128, 1024], BF16, tag="eAB",
                                        name=f"e{g}_{qc}_{kc}")
                        nc.scalar.activation(out=eAB, in_=sAB, func=AF.Exp,
                                             scale=0.125)
                        if kc == 0 and pending[0] is not None:
                            pending[0]()
                            pending[0] = None
                        # ctx runs one kc behind so the PE never waits on exp
                        if prev is not None:
                            pe, pkc = prev
                            nc.tensor.matmul(cA, vpA[pkc], pe[:, 0:512],
                                             start=(pkc == 0), stop=False)
                            nc.tensor.matmul(cB, vpB[pkc], pe[:, 512:1024],
                                             start=(pkc == 0), stop=False)
                        prev = (eAB, kc)
                    pe, pkc = prev
                    nc.tensor.matmul(cA, vpA[pkc], pe[:, 0:512],
                                     start=False, stop=True)
                    nc.tensor.matmul(cB, vpB[pkc], pe[:, 512:1024],
                                     start=False, stop=True)
                    pending[0] = epilogue(qc, cA, cB)
                return pending[0]

            pending = [None]
            for g in range(NG):
                proj_group(g, pending)
                pending = [attn_group(g, pending[0])]
            pending[0]()

    nc.finalize()
    return nc


def _get_nc(cap, zero_col0):
    key = (cap, zero_col0)
    if key not in _CACHE:
        _CACHE[key] = _build(cap, zero_col0)
    return _CACHE[key]


def _pack_queries(am_row, cap):
    """Build per-512-chunk packed index lists for one batch.

    Packed column c*cap+s <- query (c*512 + idx[c][s]).  Column 0 is
    always reserved and zeroed on the device; every masked query position
    takes its output from column 0 on the host (a zeroed q column yields
    the uniform-softmax result, identical for all masked queries).

    Returns (fits, wrapped_idx [SC*128, cap//16] int16, scatter info).
    """
    masked = np.where(am_row == 0)[0]
    cols = []          # packed column (valid entries, in order)
    qpos = []          # matching global query index
    wrapped = np.zeros((SC, 128, cap // 16), dtype=np.int16)
    for c in range(SC):
        lo = c * 512
        un = np.where(am_row[lo:lo + 512] == 1)[0]    # local indices
        reserve = 1 if c == 0 else 0
        if len(un) + reserve > cap:
            return False, None, None
        idx = np.zeros(cap, dtype=np.int16)           # pad/rep = 0 (valid)
        idx[reserve:reserve + len(un)] = un
        # wrapped layout: index j lives at [j % 16, j // 16], replicated
        # into each 16-partition block
        wrapped[c] = np.tile(idx.reshape(cap // 16, 16).T, (8, 1))
        cols.extend(c * cap + reserve + i for i in range(len(un)))
        qpos.extend(lo + int(u) for u in un)
    info = {
        "cols": np.asarray(cols, dtype=np.int64),
        "qpos": np.asarray(qpos, dtype=np.int64),
        "masked": masked,
    }
    return True, wrapped.reshape(SC * 128, cap // 16), info


def _shard_inputs(hidden_states, attention_mask, Wq, bq, Wk, bk):
    hs = np.asarray(hidden_states, dtype=np.float32)
    am = np.asarray(attention_mask)
    Wq = np.asarray(Wq, dtype=np.float32)
    Wk = np.asarray(Wk, dtype=np.float32)
    bq = np.asarray(bq, dtype=np.float32)
    bk = np.asarray(bk, dtype=np.float32)
    BF = ml_dtypes.bfloat16

    # pick the packed capacity: 384/chunk normally (random ~50% mask);
    # escalate if some chunk has too many unmasked queries
    packs = None
    for cap in (384, 512, 640):
        packs = []
        for b in range(B):
            ok, wrapped, info = _pack_queries(am[b], cap)
            if not ok:
                packs = None
                break
            packs.append((wrapped, info))
        if packs is not None:
            break
    assert packs is not None

    _META.clear()
    _META["cap"] = cap
    _META["zero_col0"] = True
    _META["packs"] = packs

    xts = [np.ascontiguousarray(
        hs[b].T.astype(BF).reshape(8, 128, SC, 512).transpose(2, 0, 1, 3)
        .reshape(SC * 8 * 128, 512)) for b in range(B)]

    in_maps = []
    for c in range(NCORES):
        b = c // (NCORES // B)
        hg = c % (NCORES // B)
        cols = slice(hg * 2 * 128, (hg + 1) * 2 * 128)

        def _tile_w(W):
            return np.ascontiguousarray(
                W[:, cols].astype(BF).reshape(8, 128, NG, 128)
                .transpose(2, 0, 1, 3).reshape(NG * 8 * 128, 128))
        in_maps.append({
            "xt": xts[b],
            "wq": _tile_w(Wq),
            "wk": _tile_w(Wk),
            "bq": np.ascontiguousarray(bq[cols]),
            "bk": np.ascontiguousarray(bk[cols]),
            "qidx": packs[b][0],
        })
    return in_maps


def _gather(results):
    cap = _META["cap"]
    NQ = cap * SC
    full = np.empty((B, S, D), dtype=np.float32)
    for c in range(NCORES):
        b = c // (NCORES // B)
        hg = c % (NCORES // B)
        _, info = _META["packs"][b]
        r = results[c]["out"].reshape(HPC, HD, NQ)
        for h in range(HPC):
            col = hg * 2 * 128 + h * 64
            blk = full[b, :, col:col + 64]
            blk[info["qpos"], :] = r[h][:, info["cols"]].T
            if len(info["masked"]):
                blk[info["masked"], :] = r[h][:, 0]
    return full


def run_sharded(in_maps, **kw):
    from concourse.bass_utils import run_bass_kernel_spmd
    nc = _get_nc(_META["cap"], _META["zero_col0"])
    return run_bass_kernel_spmd(nc, in_maps, list(range(NCORES)), **kw)


def kernel(hidden_states, attention_mask, Wq, bq, Wk, bk):
    in_maps = _shard_inputs(hidden_states, attention_mask, Wq, bq, Wk, bk)
    res = run_sharded(in_maps)
    return _gather(res.results)


# revision 22
# speedup vs baseline: 1.0846x; 1.0649x over previous
"""Bass/Tile TRN2 kernel for nn_BertSelfAttention2 (B=2, S=2048, D=1024, H=16).

Sharding: 8 cores = 2 (batch) x 4 (head groups of 4 heads). Each core
computes Q/K projections for its 4 heads (as 2 packed pairs), the modified
attention (kt = softplus(k), v = q + k, mask on the query axis), and writes
its slice of the output.

Everything is computed in "T" orientation (scoresT[k, q]) so no large
on-device transposes are needed; the final [head*64, S] -> [S, 256]
transpose happens on the host in _gather.

The query-axis mask is exploited rather than applied: every masked query
produces the SAME output (uniform softmax = mean(v)), so only the unmasked
query columns are gathered (gpsimd ap_gather) into a packed set, plus one
reserved zeroed column whose result the host broadcasts to all masked
positions.  With ~50% masked queries this cuts scores/exp/ctx work by 25%
(4x512 -> 4x384 packed columns).

All matmul operands are bf16 (fp32r streams ~2.3 cycles/col on TRN2's PE,
bf16 streams 1); accumulation stays fp32 in PSUM.  kt = ln(exp(k)+1) runs
entirely under one activation table (natural_log_exp_and_others, loaded
explicitly once) so the scalar engine never thrashes table reloads.  The
projections of group g+1 are interleaved into attention of group g so the
PE and ACT engines both stay fed (also keeps the PE activity monitor from
down-clocking 2.4 -> 1.2 GHz).  Inputs are shipped pre-tiled and packed so
the whole input load is 8 large contiguous DMAs.
"""
import sys

if "/opt/trn_rl_repo" not in sys.path:
    sys.path.insert(0, "/opt/trn_rl_repo")

import numpy as np
import ml_dtypes

B, S, D = 2, 2048, 1024
H = 16
HD = 64
NCORES = 8
HPC = H // (NCORES // B)     # heads per core = 4
NG = HPC // 2                # head-pair groups per core = 2
SC = 4                       # 512-wide seq chunks
KC = S // 128                # 16 key chunks

_CACHE = {}
_META = {}


def _build(cap, zero_col0):
    """cap: packed query columns kept per 512-seq chunk (%16==0; 384
    normally).  zero_col0: zero packed column 0 (the masked-query rep)."""
    import concourse.tile as tile
    from concourse import bacc, mybir
    from concourse.hw_specs import get_activation_tables

    F32 = mybir.dt.float32
    BF16 = mybir.dt.bfloat16
    I16 = mybir.dt.int16
    AF = mybir.ActivationFunctionType

    NQ = cap * SC                # packed query columns (1536 normally)
    SQC = NQ // 512              # packed 512-col matmul chunks (3 normally)
    assert NQ % 512 == 0 and cap % 16 == 0

    nc = bacc.Bacc(None, target_bir_lowering=False, debug=False)

    # pre-tiled packed operands: each load is one big contiguous DMA
    xt = nc.declare_dram_parameter("xt", [SC * 128, 8 * 512], BF16, isOutput=False)
    wq = nc.declare_dram_parameter("wq", [NG * 128, 8 * 128], BF16, isOutput=False)
    wk = nc.declare_dram_parameter("wk", [NG * 128, 8 * 128], BF16, isOutput=False)
    bq = nc.declare_dram_parameter("bq", [2 * 128], F32, isOutput=False)
    bk = nc.declare_dram_parameter("bk", [2 * 128], F32, isOutput=False)
    qidx = nc.declare_dram_parameter("qidx", [128, SC * (cap // 16)], I16,
                                     isOutput=False)
    out = nc.declare_dram_parameter("out", [HPC * HD, NQ], F32, isOutput=True)

    with tile.TileContext(nc) as tc:
        with tc.tile_pool(name="consts", bufs=1) as consts, \
             tc.tile_pool(name="big", bufs=1) as big, \
             tc.tile_pool(name="tmp", bufs=2) as tmp, \
             tc.tile_pool(name="expp", bufs=3) as expp, \
             tc.tile_pool(name="ep", bufs=2) as ep, \
             tc.tile_pool(name="ps_s", bufs=2, space="PSUM") as ps_s, \
             tc.tile_pool(name="ps_c", bufs=1, space="PSUM") as ps_c, \
             tc.tile_pool(name="ps_m", bufs=2, space="PSUM") as ps_m:

            # load the Exp+Ln activation table once, explicitly: with both
            # functions resident the act-table pass inserts no reloads
            if True:  # BISECT: manual combined-table load
                tabs = list(get_activation_tables(nc.m.arch))
                nl_exp_id = tabs.index("natural_log_exp_and_others")
                nc.scalar.add_instruction(mybir.InstLoadActFuncSet(
                    name=nc.get_next_instruction_name(), ins=[], outs=[],
                    act_func_set_id=nl_exp_id))

            # ---- input loads: 8 large DMAs on sync, 5 tiny on gpsimd ----
            wq_t = [consts.tile([128, 1024], BF16, tag=f"wq{g}", name=f"wq{g}")
                    for g in range(NG)]
            wk_t = [consts.tile([128, 1024], BF16, tag=f"wk{g}", name=f"wk{g}")
                    for g in range(NG)]
            xt_t = [big.tile([128, 4096], BF16, tag=f"xt{scc}", name=f"xt{scc}")
                    for scc in range(SC)]
            qidx_t = consts.tile([128, SC * (cap // 16)], I16, tag="qi",
                                 name="qi")

            nc.sync.dma_start(out=wq_t[0], in_=wq[0:128, :])
            nc.sync.dma_start(out=xt_t[0], in_=xt[0:128, :])
            nc.sync.dma_start(out=wk_t[0], in_=wk[0:128, :])
            nc.sync.dma_start(out=xt_t[1], in_=xt[128:256, :])
            nc.sync.dma_start(out=wq_t[1], in_=wq[128:256, :])
            nc.sync.dma_start(out=wk_t[1], in_=wk[128:256, :])
            nc.sync.dma_start(out=xt_t[2], in_=xt[256:384, :])
            nc.sync.dma_start(out=xt_t[3], in_=xt[384:512, :])

            bq_t, bk_t = [], []
            for g in range(NG):
                bqt = consts.tile([128, 1], F32, tag=f"bq{g}", name=f"bq{g}")
                nc.gpsimd.dma_start(
                    out=bqt,
                    in_=bq[g * 128:(g + 1) * 128].rearrange("(p o) -> p o", o=1))
                bq_t.append(bqt)
                bkt = consts.tile([128, 1], F32, tag=f"bk{g}", name=f"bk{g}")
                nc.gpsimd.dma_start(
                    out=bkt,
                    in_=bk[g * 128:(g + 1) * 128].rearrange("(p o) -> p o", o=1))
                bk_t.append(bkt)
            nc.gpsimd.dma_start(out=qidx_t, in_=qidx[:, :])

            # identity for the small V transposes
            from concourse.masks import make_identity
            ident = consts.tile([128, 128], F32)
            make_identity(nc, ident)

            # ---- persistent activations ----
            tqf = [big.tile([128, S], F32, tag=f"tqf{g}", name=f"tqf{g}")
                   for g in range(NG)]
            qg = [big.tile([128, NQ], F32, tag=f"qg{g}", name=f"qg{g}")
                  for g in range(NG)]
            # qtp: packed q per head, bf16; the other head's rows stay zero
            # so the scores matmuls run a full K=128 contraction
            qtp = [[big.tile([128, NQ], BF16, tag=f"qtp{g}_{hh}",
                             name=f"qtp{g}_{hh}") for hh in range(2)]
                   for g in range(NG)]
            for g in range(NG):
                nc.vector.memset(qtp[g][0][64:128, :], 0.0)
                nc.vector.memset(qtp[g][1][0:64, :], 0.0)
            kt = [[big.tile([128, 512], BF16, tag=f"kt{g}_{sc}",
                            name=f"kt{g}_{sc}") for sc in range(SC)]
                  for g in range(NG)]
            # vp: v^T per head per key chunk + a ones column (denominator)
            vp = [[big.tile([128, 65], BF16, tag=f"vp{h}_{kc}",
                            name=f"vp{h}_{kc}") for kc in range(KC)]
                  for h in range(HPC)]

            def proj_chunks(g):
                """Generator: emits group g's projection; yields after every
                PE instruction so attention of group g-1 can interleave."""
                for sc in range(SC):
                    ssl = slice(sc * 512, (sc + 1) * 512)
                    csl = slice(sc * cap, (sc + 1) * cap)
                    isl = slice(sc * (cap // 16), (sc + 1) * (cap // 16))
                    pq = ps_m.tile([128, 512], F32, tag="ep", name=f"pq{g}_{sc}")
                    for dc in range(8):
                        nc.tensor.matmul(pq[:, 0:512],
                                         wq_t[g][:, dc * 128:(dc + 1) * 128],
                                         xt_t[sc][:, dc * 512:(dc + 1) * 512],
                                         start=(dc == 0), stop=(dc == 7))
                        yield
                    pk = ps_m.tile([128, 512], F32, tag="ep", name=f"pk{g}_{sc}")
                    for dc in range(8):
                        nc.tensor.matmul(pk[:, 0:512],
                                         wk_t[g][:, dc * 128:(dc + 1) * 128],
                                         xt_t[sc][:, dc * 512:(dc + 1) * 512],
                                         start=(dc == 0), stop=(dc == 7))
                        yield
                    nc.vector.tensor_scalar_add(tqf[g][:, ssl], pq[:, 0:512],
                                                bq_t[g])
                    tk = tmp.tile([128, 512], F32, tag="tk", name=f"tk{g}_{sc}")
                    nc.vector.tensor_scalar_add(tk, pk[:, 0:512], bk_t[g])
                    # kt = softplus(k) = ln(exp(k) + 1), one table resident
                    te = tmp.tile([128, 512], F32, tag="te", name=f"te{g}_{sc}")
                    nc.scalar.activation(out=te, in_=tk, func=AF.Exp)
                    nc.scalar.activation(out=kt[g][sc], in_=te,
                                         func=AF.Ln, bias=1.0)
                    # v = q + k
                    vts = tmp.tile([128, 512], F32, tag="vts",
                                   name=f"vts{g}_{sc}")
                    nc.vector.tensor_add(vts, tqf[g][:, ssl], tk)
                    # packed-q gather for this chunk (gpsimd)
                    nc.gpsimd.ap_gather(
                        qg[g][:, csl], tqf[g][:, ssl], qidx_t[:, isl],
                        channels=128, num_elems=512, d=1, num_idxs=cap)
                    nc.vector.tensor_copy(qtp[g][0][0:64, csl],
                                          qg[g][0:64, csl])
                    nc.vector.tensor_copy(qtp[g][1][64:128, csl],
                                          qg[g][64:128, csl])
                    # v^T: one [128,128] PE transpose per key chunk serves
                    # both heads
                    for jj in range(4):
                        j = sc * 4 + jj
                        pv = ps_m.tile([128, 128], F32, tag="ep",
                                       name=f"pv{g}_{j}")
                        nc.tensor.transpose(pv, vts[:, jj * 128:(jj + 1) * 128],
                                            ident)
                        yield
                        for hh in range(2):
                            h = g * 2 + hh
                            nc.vector.tensor_copy(vp[h][j][:, 0:64],
                                                  pv[:, hh * 64:(hh + 1) * 64])
                            nc.vector.memset(vp[h][j][:, 64:65], 1.0)
                if zero_col0:
                    nc.vector.memset(qtp[g][0][0:64, 0:1], 0.0)
                    nc.vector.memset(qtp[g][1][64:128, 0:1], 0.0)

            def drain(gen):
                if gen is not None:
                    next(gen, None)

            def attn_group(g, gen):
                """Attention for group g; drains proj of group g+1 between
                matmuls to keep the PE fed while the ACT engine exps."""
                vpA = vp[g * 2]
                vpB = vp[g * 2 + 1]
                ktg = kt[g]
                for qc in range(SQC):
                    qsl = slice(qc * 512, (qc + 1) * 512)
                    cA = ps_c.tile([65, 512], F32, tag="cA", name=f"cA{g}_{qc}")
                    cB = ps_c.tile([65, 512], F32, tag="cB", name=f"cB{g}_{qc}")
                    prev = None
                    for kc in range(KC):
                        sAB = ps_s.tile([128, 1024], F32, tag="sAB",
                                        name=f"s{g}_{qc}_{kc}")
                        lhs = ktg[kc // 4][:, (kc % 4) * 128:(kc % 4 + 1) * 128]
                        nc.tensor.matmul(sAB[:, 0:512], lhs,
                                         qtp[g][0][:, qsl],
                                         start=True, stop=True)
                        nc.tensor.matmul(sAB[:, 512:1024], lhs,
                                         qtp[g][1][:, qsl],
                                         start=True, stop=True)
                        eAB = expp.tile([128, 1024], BF16, tag="eAB",
                                        name=f"e{g}_{qc}_{kc}")
                        nc.scalar.activation(out=eAB, in_=sAB, func=AF.Exp,
                                             scale=0.125)
                        drain(gen)
                        # ctx runs one kc behind so the PE never waits on exp
                        if prev is not None:
                            pe, pkc = prev
                            nc.tensor.matmul(cA, vpA[pkc], pe[:, 0:512],
                                             start=(pkc == 0), stop=False)
                            nc.tensor.matmul(cB, vpB[pkc], pe[:, 512:1024],
                                             start=(pkc == 0), stop=False)
                        drain(gen)
                        prev = (eAB, kc)
                    pe, pkc = prev
                    nc.tensor.matmul(cA, vpA[pkc], pe[:, 0:512],
                                     start=False, stop=True)
                    nc.tensor.matmul(cB, vpB[pkc], pe[:, 512:1024],
                                     start=False, stop=True)
                    # epilogue (no PE ops): copy ctx+denominator to SBUF
                    # right away (frees the PSUM accumulator so the next qc
                    # never stalls), then the slow DVE reciprocal, gpsimd
                    # broadcast and multiply all run off the critical path
                    for hh, c in ((0, cA), (1, cB)):
                        cs = ep.tile([65, 512], F32, tag=f"cs{hh}",
                                     name=f"cs{g}_{qc}_{hh}")
                        nc.vector.tensor_copy(cs, c)
                        rcp = ep.tile([1, 512], F32, tag=f"r{hh}",
                                      name=f"r{g}_{qc}_{hh}")
                        nc.vector.reciprocal(rcp, cs[64:65, :])
                        bc = ep.tile([64, 512], F32, tag=f"b{hh}",
                                     name=f"b{g}_{qc}_{hh}")
                        nc.gpsimd.partition_broadcast(bc, rcp[0:1, :])
                        cf = ep.tile([64, 512], F32, tag=f"cf{hh}",
                                     name=f"cf{g}_{qc}_{hh}")
                        nc.vector.tensor_mul(cf, cs[0:64, :], bc)
                        row = (g * 2 + hh) * 64
                        nc.sync.dma_start(
                            out=out[row:row + 64, qc * 512:(qc + 1) * 512],
                            in_=cf)

            gen0 = proj_chunks(0)
            for _ in gen0:
                pass
            for g in range(NG):
                gen_next = proj_chunks(g + 1) if g + 1 < NG else None
                attn_group(g, gen_next)
                if gen_next is not None:
                    for _ in gen_next:
                        pass

    nc.finalize()
    return nc


def _get_nc(cap, zero_col0):
    key = (cap, zero_col0)
    if key not in _CACHE:
        _CACHE[key] = _build(cap, zero_col0)
    return _CACHE[key]


def _pack_queries(am_row, cap):
    """Build per-512-chunk packed index lists for one batch.

    Packed column c*cap+s <- query (c*512 + idx[c][s]).  Column 0 is
    always reserved and zeroed on the device; every masked query position
    takes its output from column 0 on the host (a zeroed q column yields
    the uniform-softmax result, identical for all masked queries).

    Returns (fits, wrapped_idx [128, SC*cap//16] int16, scatter info).
    """
    masked = np.where(am_row == 0)[0]
    cols = []          # packed column (valid entries, in order)
    qpos = []          # matching global query index
    wrapped = np.zeros((SC, 128, cap // 16), dtype=np.int16)
    for c in range(SC):
        lo = c * 512
        un = np.where(am_row[lo:lo + 512] == 1)[0]    # local indices
        reserve = 1 if c == 0 else 0
        if len(un) + reserve > cap:
            return False, None, None
        idx = np.zeros(cap, dtype=np.int16)           # pad/rep = 0 (valid)
        idx[reserve:reserve + len(un)] = un
        # wrapped layout: index j lives at [j % 16, j // 16], replicated
        # into each 16-partition block
        wrapped[c] = np.tile(idx.reshape(cap // 16, 16).T, (8, 1))
        cols.extend(c * cap + reserve + i for i in range(len(un)))
        qpos.extend(lo + int(u) for u in un)
    info = {
        "cols": np.asarray(cols, dtype=np.int64),
        "qpos": np.asarray(qpos, dtype=np.int64),
        "masked": masked,
    }
    return True, wrapped.transpose(1, 0, 2).reshape(128, SC * (cap // 16)), info


def _shard_inputs(hidden_states, attention_mask, Wq, bq, Wk, bk):
    hs = np.asarray(hidden_states, dtype=np.float32)
    am = np.asarray(attention_mask)
    Wq = np.asarray(Wq, dtype=np.float32)
    Wk = np.asarray(Wk, dtype=np.float32)
    bq = np.asarray(bq, dtype=np.float32)
    bk = np.asarray(bk, dtype=np.float32)
    BF = ml_dtypes.bfloat16

    # packed capacity: 384/chunk for the random ~50% mask; escalate if a
    # chunk has too many unmasked queries (640 always fits: 512+1 <= 640)
    packs = None
    for cap in (384, 512, 640):
        packs = []
        for b in range(B):
            ok, wrapped, info = _pack_queries(am[b], cap)
            if not ok:
                packs = None
                break
            packs.append((wrapped, info))
        if packs is not None:
            break
    assert packs is not None

    _META.clear()
    _META["cap"] = cap
    _META["zero_col0"] = True
    _META["packs"] = packs

    # X^T packed per seq chunk: [sc][p, dc*512+s] = X[b, sc*512+s, dc*128+p]
    xts = [np.ascontiguousarray(
        hs[b].T.astype(BF).reshape(8, 128, SC, 512).transpose(2, 1, 0, 3)
        .reshape(SC * 128, 8 * 512)) for b in range(B)]

    in_maps = []
    for c in range(NCORES):
        b = c // (NCORES // B)
        hg = c % (NCORES // B)
        cols = slice(hg * 2 * 128, (hg + 1) * 2 * 128)

        def _tile_w(W):
            # [g][p, dc*128+j] = W[dc*128+p, cols[g*128+j]]
            return np.ascontiguousarray(
                W[:, cols].astype(BF).reshape(8, 128, NG, 128)
                .transpose(2, 1, 0, 3).reshape(NG * 128, 8 * 128))
        in_maps.append({
            "xt": xts[b],
            "wq": _tile_w(Wq),
            "wk": _tile_w(Wk),
            "bq": np.ascontiguousarray(bq[cols]),
            "bk": np.ascontiguousarray(bk[cols]),
            "qidx": packs[b][0],
        })
    return in_maps


def _gather(results):
    cap = _META["cap"]
    NQ = cap * SC
    full = np.empty((B, S, D), dtype=np.float32)
    for c in range(NCORES):
        b = c // (NCORES // B)
        hg = c % (NCORES // B)
        _, info = _META["packs"][b]
        r = results[c]["out"].reshape(HPC, HD, NQ)
        for h in range(HPC):
            col = hg * 2 * 128 + h * 64
            blk = full[b, :, col:col + 64]
            blk[info["qpos"], :] = r[h][:, info["cols"]].T
            if len(info["masked"]):
                blk[info["masked"], :] = r[h][:, 0]
    return full


def run_sharded(in_maps, **kw):
    from concourse.bass_utils import run_bass_kernel_spmd
    nc = _get_nc(_META["cap"], _META["zero_col0"])
    return run_bass_kernel_spmd(nc, in_maps, list(range(NCORES)), **kw)


def kernel(hidden_states, attention_mask, Wq, bq, Wk, bk):
    in_maps = _shard_inputs(hidden_states, attention_mask, Wq, bq, Wk, bk)
    res = run_sharded(in_maps)
    return _gather(res.results)
